# revision 2
# baseline (speedup 1.0000x reference)
"""Bundle-adjustment forward projection on 8 Trainium2 NeuronCores.

reference:  R = euler_to_matrix(euler_angles)            [V,3,3]
            pc = einsum('nj,vij->vni', points3d, R) + t  [V,N,3]
            Zc = min(pc_z, -1e-4)
            u = -f*Xc/Zc + CX ; v = f*Yc/Zc + CY         -> [V,N,2]

Sharding: N=200000 points split across 8 cores (25000 each); every core
computes all V=128 views so the SBUF partition dim is fully used.  The host
folds f/CX/CY/translations into three linear maps over homogeneous points
(U = p.Wu, V = p.Wv, Z' = p.Wz with u = U/Z', v = V/Z'), exact when the Z
clamp never fires (host-verified bound per call; a clamped fallback module
is built otherwise).

Device pipeline per 500-point chunk:
  * three CONCURRENT K=5 matmuls in separate PE row groups (tile_position
    (0,0)/(32,0)/(64,0)) -> U, V chunk banks + z pair bank in PSUM.
    Weights single-bf16 + hi/lo bias rows; points single-bf16 (input blob
    0.75 MB/core, streamed in 4 pieces, first on the HWDGE queue).
  * ACT: one Reciprocal per PAIR of chunks ([128,1000] PSUM->SBUF fp16),
    emitted before the uv matmuls so it runs ahead of its consumers.
  * drain: output is fp16 (error ~1.5e-3 of scale vs the 2e-2 gate),
    halving the dominant HBM store stream to 12.8 MB/core.  Most pairs
    drain via DVE tensor_tensor (PSUM f32 x recip, 1x); every 3rd pair is
    'staged': ACT copies U,V to SBUF fp16 and one pair-wide DVE TT runs in
    2x_1p packed mode.  Staged TT+store are emitted one pair late so the
    DVE strict-FIFO queue never head-of-line blocks on the copies.
  * one 512 KB store per pair on the sync queue (final store split).

Walrus in this build accepts at most ONE semaphore wait per instruction:
TileContext's tail drain is patched to split its waits into one-wait nops,
and a serialized-BIR rewriter injects same-engine NoOps for any remaining
multi-wait instruction.

Measured: ~86.4 us HW exec (min over reps; engine busy: DVE ~60us drain,
ACT ~49us recips+copies, PE ~46us, stores ~36us/engine), rel err 1.5e-3.
"""

import numpy as np
from contextlib import ExitStack

import concourse.bass as bass
import concourse.tile as tile
from concourse import mybir
from concourse.bass_utils import run_bass_kernel_spmd
from concourse.vector_clock import ScopedClock, VectorClock

CX = 512.0
CY = 512.0
Z_MAX = -1e-4

N_CORES = 8
N_POINTS = 200000
N_VIEWS = 128
NPC = N_POINTS // N_CORES          # 25000 points per core
CHUNK = 500                        # one PSUM bank per matmul output
CHUNKS = NPC // CHUNK              # 50
PAIRS = CHUNKS // 2                # 25
KR = 5                             # lhsT rows: w0,w1,w2,b_hi,b_lo
WCOLS = N_VIEWS                    # 128 weight columns per group
BLOBC = WCOLS + NPC                # per-group blob columns

# pair classes: 'A' DVE-direct TT from PSUM (1x), 'B' ACT copies both chunks
# to SBUF fp16 + ONE pair TT on DVE at 2x_1p, 'C' same staging + GPSIMD TT.
# Tuned empirically.
# Pattern: every 3rd pair staged, alternating B/C, so ACT (recip + copies)
# stays just under DVE per 3-pair window.  Pairs 0-1 and the final pair
# stay A (pipeline fill / fast drain at the tail).
PATTERN = ['A', 'A', 'B']

F32 = mybir.dt.float32
F16 = mybir.dt.float16
BF16 = mybir.dt.bfloat16

# input pieces, in pairs: weights+pair0 first so matmuls start early
PIECE_PAIRS = [(0, 5), (5, 13), (13, 21), (21, PAIRS)]


def _pair_classes():
    cls = []
    k = 0
    for j in range(PAIRS):
        if j < 2 or j == PAIRS - 1:
            cls.append('A')
        else:
            cls.append(PATTERN[k % len(PATTERN)])
            k += 1
    return cls


# ---------------------------------------------------------------------------
# Tile tail-drain workaround: this walrus build only accepts ONE semaphore
# wait per CTRL instruction, but TileContext puts every outstanding proc's
# wait on the single tail Drain.  Emit one-wait nops first instead.
# ---------------------------------------------------------------------------
def _split_drain_and_barrier(self, tick_clock, wait_clock):
    gc = tick_clock.global_clock
    n = len(gc)
    for p in range(n):
        if gc[p] > 0:
            vec = [0] * n
            vec[p] = gc[p]
            nop = self.nc.sync.nop()
            wait_clock.add_sem_waits(nop.ins, ScopedClock({None: VectorClock(vec)}))
    self.nc.sync.drain()
    self.nc.all_engine_barrier()
    assert self.sems is not None
    popped = self.nc._tile_sem_poison_stack.pop()
    assert popped is self._sem_poison
    self.nc.clear_and_free_semaphores(list(self.sems.allocated().values()))
    self.nc.all_engine_barrier()


tile.TileContext._drain_and_barrier = _split_drain_and_barrier


def _legalize_waits(bir: bytes) -> bytes:
    """Split every multi-wait instruction by injecting same-engine NoOps
    (each carrying one wait) immediately before it."""
    import json as _json

    d = _json.loads(bir)
    ctr = 0
    for f in d["functions"]:
        for b in f["blocks"]:
            newl = []
            for inst in b["instructions"]:
                si = inst.get("sync_info")
                w = (si or {}).get("on_wait") or []
                if len(w) > 1:
                    for extra in w[:-1]:
                        ctr += 1
                        newl.append(
                            {
                                "debug": inst.get("debug", 0),
                                "engine": inst["engine"],
                                "ins": [],
                                "outs": [],
                                "name": f"I-wfix{ctr}",
                                "opcode": "NoOp",
                                "sync_info": {"on_update": [], "on_wait": [extra]},
                            }
                        )
                    si["on_wait"] = [w[-1]]
                newl.append(inst)
            b["instructions"] = newl
    return _json.dumps(d).encode()


def _install_wait_legalizer(nc):
    orig = nc.to_json_bytes

    def to_json_bytes_fixed():
        return _legalize_waits(orig())

    nc.to_json_bytes = to_json_bytes_fixed
    return nc


# ---------------------------------------------------------------------------
# Host-side math
# ---------------------------------------------------------------------------
def _euler_to_matrix(e):
    x, y, z = e[:, 0], e[:, 1], e[:, 2]
    c1, s1 = np.cos(x), np.sin(x)
    c2, s2 = np.cos(y), np.sin(y)
    c3, s3 = np.cos(z), np.sin(z)
    zero = np.zeros_like(x)
    one = np.ones_like(x)
    Rx = np.stack([one, zero, zero, zero, c1, -s1, zero, s1, c1], -1).reshape(-1, 3, 3)
    Ry = np.stack([c2, zero, s2, zero, one, zero, -s2, zero, c2], -1).reshape(-1, 3, 3)
    Rz = np.stack([c3, -s3, zero, s3, c3, zero, zero, zero, one], -1).reshape(-1, 3, 3)
    return Rx @ Ry @ Rz


def _fold_weights(euler_angles, translations, focal_length, clamp):
    """Three [KR, V] bf16 lhsT blocks: rows [w0,w1,w2,b_hi,b_lo]."""
    import ml_dtypes

    R = _euler_to_matrix(euler_angles.astype(np.float64))
    t = translations.astype(np.float64)
    f = float(focal_length[0])
    r0, r1, r2 = R[:, 0, :], R[:, 1, :], R[:, 2, :]
    tx, ty, tz = t[:, 0], t[:, 1], t[:, 2]

    if clamp:
        wU = f * r0
        bU = f * tx
        wV = -f * r1
        bV = -f * ty
    else:
        wU = f * r0 - CX * r2
        bU = f * tx - CX * tz
        wV = -f * r1 - CY * r2
        bV = -f * ty - CY * tz
    wZ = -r2
    bZ = -tz

    def pack(w, b):
        w16 = w.astype(ml_dtypes.bfloat16)
        b_hi = b.astype(ml_dtypes.bfloat16)
        b_lo = (b - b_hi.astype(np.float64)).astype(ml_dtypes.bfloat16)
        return np.concatenate([w16.T, b_hi[None, :], b_lo[None, :]], axis=0)

    return pack(wU, bU), pack(wV, bV), pack(wZ, bZ)


# ---------------------------------------------------------------------------
# Bass module
# ---------------------------------------------------------------------------
def _build_module(clamp):
    nc = bass.Bass()
    blob = nc.declare_dram_parameter("blob", [KR, 3 * BLOBC], BF16, isOutput=False)
    out = nc.declare_dram_parameter("out", [N_VIEWS, PAIRS * 4 * CHUNK], F16,
                                    isOutput=True)

    classes = _pair_classes()

    with tile.TileContext(nc) as tc, ExitStack() as ctx:
        const_pool = ctx.enter_context(tc.tile_pool(name="const", bufs=1))
        uv_pool = ctx.enter_context(tc.tile_pool(name="uvps", bufs=2, space="PSUM"))
        z_pool = ctx.enter_context(tc.tile_pool(name="zps", bufs=2, space="PSUM"))
        rz_pool = ctx.enter_context(tc.tile_pool(name="rz", bufs=4))
        sbuv_pool = ctx.enter_context(tc.tile_pool(name="sbuv", bufs=4))
        out_pool = ctx.enter_context(tc.tile_pool(name="out", bufs=4))
        warm_pool = ctx.enter_context(tc.tile_pool(name="warm", bufs=1))

        # blob tile: groups u/v/z at partition bases 0/32/64, KR rows each
        btile = const_pool.tile([96, BLOBC], BF16, tag="blob")

        def load_piece(pi):
            if pi >= len(PIECE_PAIRS):
                return
            p0, p1 = PIECE_PAIRS[pi]
            lo = 0 if pi == 0 else WCOLS + p0 * 2 * CHUNK
            hi = WCOLS + p1 * 2 * CHUNK
            # piece 0 rides HWDGE (sync): ~0.6us first-byte vs ~1us+Q7 queue
            # on SWDGE, so the first matmuls start earlier
            eng = nc.sync if pi == 0 else nc.gpsimd
            for g in range(3):
                eng.dma_start(
                    btile[32 * g : 32 * g + KR, lo:hi],
                    blob[:, g * BLOBC + lo : g * BLOBC + hi],
                )

        load_piece(0)
        load_piece(1)

        ACT_FN = mybir.ActivationFunctionType

        def act_direct(out_ap, in_ap, func, bias=0.0, scale=1.0, alpha=0.0):
            # same lowering as nc.scalar.activation but without the
            # Reciprocal accuracy guard (domain here is ~[1.1, 3.6])
            eng = nc.scalar
            ins = [eng.lower_ap(in_ap)]
            for val in (bias, scale, alpha):
                ins.append(mybir.ImmediateValue(dtype=mybir.dt.float32, value=val))
            return eng.add_instruction(
                mybir.InstActivation(
                    name=nc.get_next_instruction_name(),
                    func=func,
                    ins=ins,
                    outs=[eng.lower_ap(out_ap)],
                )
            )

        # pre-warm the ACT spline tables under the input transfer
        warm = warm_pool.tile([1, 2], F32, tag="warm")
        nc.vector.memset(warm[:], 1.0)
        act_direct(warm[0:1, 1:2], warm[0:1, 0:1], ACT_FN.Reciprocal)

        piece_starts = {3: 2, 11: 3}

        def rhs(c):
            return slice(WCOLS + c * CHUNK, WCOLS + (c + 1) * CHUNK)

        # deferred 'finish' closures for staged (B) pairs: the pair-TT and
        # store are emitted one pair LATE so the DVE's strict-FIFO queue
        # never head-of-line blocks on the ACT staging copies
        deferred = []

        def flush_deferred():
            while deferred:
                deferred.pop(0)()

        for j in range(PAIRS):
            if j in piece_starts:
                load_piece(piece_starts[j])

            zt = z_pool.tile([N_VIEWS, 1024], F32, tag="zt")
            rz = rz_pool.tile([N_VIEWS, 2 * CHUNK], F16, tag="rz")
            sb_out = out_pool.tile([N_VIEWS, 4 * CHUNK], F16, tag="g")
            ov = sb_out[:].rearrange("p (two h n) -> p two h n", two=2, h=2)

            # z matmuls FIRST so the pair's reciprocal runs ahead of the
            # TTs that consume it (keeps ACT off the critical path)
            for h in (0, 1):
                nc.tensor.matmul(
                    zt[:, h * 512 : h * 512 + CHUNK],
                    btile[64 : 64 + KR, 0:WCOLS],
                    btile[64 : 64 + KR, rhs(2 * j + h)],
                    tile_position=(64, 0),
                )
            zt3 = zt[:].rearrange("p (b n) -> p b n", b=2)[:, :, 0:CHUNK]
            rz3 = rz[:].rearrange("p (b n) -> p b n", b=2)
            if clamp:
                zcl = sbuv_pool.tile([N_VIEWS, 2 * CHUNK], F32, tag="zcl")
                zcl3 = zcl[:].rearrange("p (b n) -> p b n", b=2)
                nc.vector.tensor_scalar_max(zcl3, zt3, -Z_MAX)
                act_direct(rz3, zcl3, ACT_FN.Reciprocal)
            else:
                act_direct(rz3, zt3, ACT_FN.Reciprocal)

            puvs = []
            for h in (0, 1):
                c = 2 * j + h
                puv = uv_pool.tile([N_VIEWS, 1024], F32, tag="puv")
                puvs.append(puv)
                for g, dst in ((0, puv[:, 0:CHUNK]),
                               (1, puv[:, 512 : 512 + CHUNK])):
                    base = 32 * g
                    nc.tensor.matmul(
                        dst,
                        btile[base : base + KR, 0:WCOLS],
                        btile[base : base + KR, rhs(c)],
                        tile_position=(base, 0),
                    )

            kind = 'A' if clamp else classes[j]

            def store_pair(jj, tile_):
                if jj == PAIRS - 1:
                    # split the final store so the tail only waits on 256 KB
                    half = 2 * CHUNK
                    nc.sync.dma_start(
                        out[:, jj * 4 * CHUNK : jj * 4 * CHUNK + half],
                        tile_[:, 0:half])
                    nc.sync.dma_start(
                        out[:, jj * 4 * CHUNK + half : (jj + 1) * 4 * CHUNK],
                        tile_[:, half : 4 * CHUNK])
                else:
                    nc.sync.dma_start(
                        out[:, jj * 4 * CHUNK : (jj + 1) * 4 * CHUNK],
                        tile_[:])

            if kind == 'A':
                for h in (0, 1):
                    puv4 = (puvs[h][:].rearrange("p (b n) -> p b n", b=2)
                            [:, :, 0:CHUNK].unsqueeze(2))
                    rb = (rz[:, h * CHUNK : (h + 1) * CHUNK]
                          .unsqueeze(1).unsqueeze(1)
                          .broadcast_to([N_VIEWS, 2, 1, CHUNK]))
                    odst = ov[:, :, h : h + 1, :]
                    if clamp:
                        tuv = sbuv_pool.tile([N_VIEWS, 2 * CHUNK], F32,
                                             tag="tuv")
                        t4 = (tuv[:].rearrange("p (b n) -> p b n", b=2)
                              .unsqueeze(2))
                        nc.vector.tensor_tensor(t4, puv4, rb,
                                                mybir.AluOpType.mult)
                        nc.vector.tensor_scalar_add(
                            ov[:, 0:1, h : h + 1, :], t4[:, 0:1, :, :], CX)
                        nc.vector.tensor_scalar_add(
                            ov[:, 1:2, h : h + 1, :], t4[:, 1:2, :, :], CY)
                    else:
                        nc.vector.tensor_tensor(odst, puv4, rb,
                                                mybir.AluOpType.mult)
                store_pair(j, sb_out)
                flush_deferred()
            else:
                # stage both chunks to SBUF fp16 (ACT), then ONE pair-wide
                # TT: [p, uv, h, n] all step-1 fp16 -> DVE 2x_1p
                sbuv = sbuv_pool.tile([N_VIEWS, 4 * CHUNK], F16, tag="sbuv")
                sv = sbuv[:].rearrange("p (two h n) -> p two h n", two=2, h=2)
                for h in (0, 1):
                    puv4 = (puvs[h][:].rearrange("p (b n) -> p b n", b=2)
                            [:, :, 0:CHUNK].unsqueeze(2))
                    act_direct(sv[:, :, h : h + 1, :], puv4, ACT_FN.Copy)
                rbp = (rz[:].rearrange("p (h n) -> p h n", h=2)
                       .unsqueeze(1)
                       .broadcast_to([N_VIEWS, 2, 2, CHUNK]))

                def finish(jj=j, sv=sv, rbp=rbp, ovd=ov, tile_=sb_out,
                           eng=(nc.vector if kind == 'B' else nc.gpsimd)):
                    eng.tensor_tensor(ovd, sv, rbp, mybir.AluOpType.mult)
                    store_pair(jj, tile_)

                deferred.append(finish)

        flush_deferred()

    return _install_wait_legalizer(nc)


_module_cache = {}


def _get_module(clamp):
    if clamp not in _module_cache:
        _module_cache[clamp] = _build_module(clamp)
    return _module_cache[clamp]


# ---------------------------------------------------------------------------
# Entry point
# ---------------------------------------------------------------------------
def kernel(points3d, euler_angles, translations, focal_length, _trace=False):
    import ml_dtypes

    points3d = np.asarray(points3d, dtype=np.float32)
    euler_angles = np.asarray(euler_angles, dtype=np.float32)
    translations = np.asarray(translations, dtype=np.float32)
    focal_length = np.asarray(focal_length, dtype=np.float32)

    # Is the Z clamp provably inactive?  The fast path folds CX/CY into the
    # matmul, which is only exact when no point clamps.
    Rq = _euler_to_matrix(euler_angles.astype(np.float64))
    tz = translations[:, 2].astype(np.float64)
    r2n = np.linalg.norm(Rq[:, 2, :], axis=1)
    pmax = float(np.linalg.norm(points3d.astype(np.float64), axis=1).max())
    znega_lo = float((-tz - r2n * pmax).min())
    clamp = bool(znega_lo < max(-Z_MAX * 10.0, 1e-3))

    Wu, Wv, Wz = _fold_weights(euler_angles, translations, focal_length, clamp)

    pT = points3d.T.astype(ml_dtypes.bfloat16)        # [3, N]
    ones = np.ones((2, N_POINTS), dtype=ml_dtypes.bfloat16)
    pk = np.concatenate([pT, ones], axis=0)           # [KR, N]

    nc = _get_module(clamp)
    in_maps = []
    for c in range(N_CORES):
        sl = pk[:, c * NPC : (c + 1) * NPC]
        in_maps.append(
            {
                "blob": np.ascontiguousarray(
                    np.concatenate([Wu, sl, Wv, sl, Wz, sl], axis=1)
                ),
            }
        )

    res = run_bass_kernel_spmd(
        nc, in_maps, core_ids=list(range(N_CORES)), trace=_trace
    )

    full = np.empty((N_VIEWS, N_POINTS, 2), dtype=np.float32)
    for c in range(N_CORES):
        r = np.asarray(res.results[c]["out"])
        r = r.reshape(N_VIEWS, PAIRS, 2, 2, CHUNK).transpose(0, 1, 3, 4, 2)
        full[:, c * NPC : (c + 1) * NPC, :] = r.reshape(
            N_VIEWS, NPC, 2).astype(np.float32)
    if _trace:
        return full, res
    return full


# revision 3
# speedup vs baseline: 1.0335x; 1.0335x over previous
"""Bundle-adjustment forward projection on 8 Trainium2 NeuronCores.

reference:  R = euler_to_matrix(euler_angles)            [V,3,3]
            pc = einsum('nj,vij->vni', points3d, R) + t  [V,N,3]
            Zc = min(pc_z, -1e-4)
            u = -f*Xc/Zc + CX ; v = f*Yc/Zc + CY         -> [V,N,2]

Sharding: N=200000 points split across 8 cores (25000 each); every core
computes all V=128 views so the SBUF partition dim is fully used.  The host
folds f/CX/CY/translations into three linear maps over homogeneous points
(U = p.Wu, V = p.Wv, Z' = p.Wz with u = U/Z', v = V/Z'), exact when the Z
clamp never fires (host-verified bound per call; a clamped fallback module
is built otherwise).

Device pipeline per 500-point chunk:
  * three CONCURRENT K=5 matmuls in separate PE row groups (tile_position
    (0,0)/(32,0)/(64,0)) -> U, V chunk banks + z pair bank in PSUM.
    Weights single-bf16 + hi/lo bias rows; points single-bf16 (input blob
    0.75 MB/core, streamed in 4 pieces, first on the HWDGE queue).
  * ACT: one Reciprocal per PAIR of chunks ([128,1000] PSUM->SBUF fp16),
    emitted before the uv matmuls so it runs ahead of its consumers.
  * drain: output is fp16 (error ~1.5e-3 of scale vs the 2e-2 gate),
    halving the dominant HBM store stream to 12.8 MB/core.  Most pairs
    drain via DVE tensor_tensor (PSUM f32 x recip, 1x); every 3rd pair is
    'staged': ACT copies U,V to SBUF fp16 and one pair-wide DVE TT runs in
    2x_1p packed mode.  Staged TT+store are emitted one pair late so the
    DVE strict-FIFO queue never head-of-line blocks on the copies.
  * one 512 KB store per pair on the sync queue (final store split).

Walrus in this build accepts at most ONE semaphore wait per instruction:
TileContext's tail drain is patched to split its waits into one-wait nops,
and a serialized-BIR rewriter injects same-engine NoOps for any remaining
multi-wait instruction.

Measured: ~86.4 us HW exec (min over reps; engine busy: DVE ~60us drain,
ACT ~49us recips+copies, PE ~46us, stores ~36us/engine), rel err 1.5e-3.
"""

import numpy as np
from contextlib import ExitStack

import concourse.bass as bass
import concourse.tile as tile
from concourse import mybir
from concourse.bass_utils import run_bass_kernel_spmd
from concourse.vector_clock import ScopedClock, VectorClock

CX = 512.0
CY = 512.0
Z_MAX = -1e-4

N_CORES = 8
N_POINTS = 200000
N_VIEWS = 128
NPC = N_POINTS // N_CORES          # 25000 points per core
CHUNK = 500                        # one PSUM bank per matmul output
CHUNKS = NPC // CHUNK              # 50
PAIRS = CHUNKS // 2                # 25
KR = 5                             # lhsT rows: w0,w1,w2,b_hi,b_lo
WCOLS = N_VIEWS                    # 128 weight columns per group
BLOBC = WCOLS + NPC                # per-group blob columns

# pair classes: 'A' DVE-direct TT from PSUM (1x), 'B' ACT copies both chunks
# to SBUF fp16 + ONE pair TT on DVE at 2x_1p, 'C' same staging + GPSIMD TT.
# Tuned empirically.
# Pattern: every 3rd pair staged, alternating B/C, so ACT (recip + copies)
# stays just under DVE per 3-pair window.  Pairs 0-1 and the final pair
# stay A (pipeline fill / fast drain at the tail).
PATTERN = ['A', 'A', 'B']

F32 = mybir.dt.float32
F16 = mybir.dt.float16
BF16 = mybir.dt.bfloat16

# input pieces, in pairs: weights+pair0 first so matmuls start early
PIECE_PAIRS = [(0, 1), (1, 7), (7, 13), (13, 19), (19, PAIRS)]


def _pair_classes():
    cls = []
    k = 0
    for j in range(PAIRS):
        if j < 2 or j == PAIRS - 1:
            cls.append('A')
        else:
            cls.append(PATTERN[k % len(PATTERN)])
            k += 1
    return cls


# ---------------------------------------------------------------------------
# Tile tail-drain workaround: this walrus build only accepts ONE semaphore
# wait per CTRL instruction, but TileContext puts every outstanding proc's
# wait on the single tail Drain.  Emit one-wait nops first instead.
# ---------------------------------------------------------------------------
def _split_drain_and_barrier(self, tick_clock, wait_clock):
    gc = tick_clock.global_clock
    n = len(gc)
    for p in range(n):
        if gc[p] > 0:
            vec = [0] * n
            vec[p] = gc[p]
            nop = self.nc.sync.nop()
            wait_clock.add_sem_waits(nop.ins, ScopedClock({None: VectorClock(vec)}))
    self.nc.sync.drain()
    self.nc.all_engine_barrier()
    assert self.sems is not None
    popped = self.nc._tile_sem_poison_stack.pop()
    assert popped is self._sem_poison
    self.nc.clear_and_free_semaphores(list(self.sems.allocated().values()))
    self.nc.all_engine_barrier()


tile.TileContext._drain_and_barrier = _split_drain_and_barrier


def _legalize_waits(bir: bytes) -> bytes:
    """Split every multi-wait instruction by injecting same-engine NoOps
    (each carrying one wait) immediately before it."""
    import json as _json

    d = _json.loads(bir)
    ctr = 0
    for f in d["functions"]:
        for b in f["blocks"]:
            newl = []
            for inst in b["instructions"]:
                si = inst.get("sync_info")
                w = (si or {}).get("on_wait") or []
                if len(w) > 1:
                    for extra in w[:-1]:
                        ctr += 1
                        newl.append(
                            {
                                "debug": inst.get("debug", 0),
                                "engine": inst["engine"],
                                "ins": [],
                                "outs": [],
                                "name": f"I-wfix{ctr}",
                                "opcode": "NoOp",
                                "sync_info": {"on_update": [], "on_wait": [extra]},
                            }
                        )
                    si["on_wait"] = [w[-1]]
                newl.append(inst)
            b["instructions"] = newl
    return _json.dumps(d).encode()


def _install_wait_legalizer(nc):
    orig = nc.to_json_bytes

    def to_json_bytes_fixed():
        return _legalize_waits(orig())

    nc.to_json_bytes = to_json_bytes_fixed
    return nc


# ---------------------------------------------------------------------------
# Host-side math
# ---------------------------------------------------------------------------
def _euler_to_matrix(e):
    x, y, z = e[:, 0], e[:, 1], e[:, 2]
    c1, s1 = np.cos(x), np.sin(x)
    c2, s2 = np.cos(y), np.sin(y)
    c3, s3 = np.cos(z), np.sin(z)
    zero = np.zeros_like(x)
    one = np.ones_like(x)
    Rx = np.stack([one, zero, zero, zero, c1, -s1, zero, s1, c1], -1).reshape(-1, 3, 3)
    Ry = np.stack([c2, zero, s2, zero, one, zero, -s2, zero, c2], -1).reshape(-1, 3, 3)
    Rz = np.stack([c3, -s3, zero, s3, c3, zero, zero, zero, one], -1).reshape(-1, 3, 3)
    return Rx @ Ry @ Rz


def _fold_weights(euler_angles, translations, focal_length, clamp):
    """Three [KR, V] bf16 lhsT blocks: rows [w0,w1,w2,b_hi,b_lo]."""
    import ml_dtypes

    R = _euler_to_matrix(euler_angles.astype(np.float64))
    t = translations.astype(np.float64)
    f = float(focal_length[0])
    r0, r1, r2 = R[:, 0, :], R[:, 1, :], R[:, 2, :]
    tx, ty, tz = t[:, 0], t[:, 1], t[:, 2]

    if clamp:
        wU = f * r0
        bU = f * tx
        wV = -f * r1
        bV = -f * ty
    else:
        wU = f * r0 - CX * r2
        bU = f * tx - CX * tz
        wV = -f * r1 - CY * r2
        bV = -f * ty - CY * tz
    wZ = -r2
    bZ = -tz

    def pack(w, b):
        w16 = w.astype(ml_dtypes.bfloat16)
        b_hi = b.astype(ml_dtypes.bfloat16)
        b_lo = (b - b_hi.astype(np.float64)).astype(ml_dtypes.bfloat16)
        return np.concatenate([w16.T, b_hi[None, :], b_lo[None, :]], axis=0)

    return pack(wU, bU), pack(wV, bV), pack(wZ, bZ)


# ---------------------------------------------------------------------------
# Bass module
# ---------------------------------------------------------------------------
def _build_module(clamp):
    nc = bass.Bass()
    blob = nc.declare_dram_parameter("blob", [KR, 3 * BLOBC], BF16, isOutput=False)
    out = nc.declare_dram_parameter("out", [N_VIEWS, PAIRS * 4 * CHUNK], F16,
                                    isOutput=True)

    classes = _pair_classes()

    with tile.TileContext(nc) as tc, ExitStack() as ctx:
        const_pool = ctx.enter_context(tc.tile_pool(name="const", bufs=1))
        uv_pool = ctx.enter_context(tc.tile_pool(name="uvps", bufs=2, space="PSUM"))
        z_pool = ctx.enter_context(tc.tile_pool(name="zps", bufs=2, space="PSUM"))
        rz_pool = ctx.enter_context(tc.tile_pool(name="rz", bufs=4))
        sbuv_pool = ctx.enter_context(tc.tile_pool(name="sbuv", bufs=4))
        out_pool = ctx.enter_context(tc.tile_pool(name="out", bufs=4))
        warm_pool = ctx.enter_context(tc.tile_pool(name="warm", bufs=1))

        # blob tile: groups u/v/z at partition bases 0/32/64, KR rows each
        btile = const_pool.tile([96, BLOBC], BF16, tag="blob")

        def load_piece(pi):
            if pi >= len(PIECE_PAIRS):
                return
            p0, p1 = PIECE_PAIRS[pi]
            lo = 0 if pi == 0 else WCOLS + p0 * 2 * CHUNK
            hi = WCOLS + p1 * 2 * CHUNK
            # piece 0 rides HWDGE (sync): ~0.6us first-byte vs ~1us+Q7 queue
            # on SWDGE, so the first matmuls start earlier
            eng = nc.sync if pi == 0 else nc.gpsimd
            for g in range(3):
                eng.dma_start(
                    btile[32 * g : 32 * g + KR, lo:hi],
                    blob[:, g * BLOBC + lo : g * BLOBC + hi],
                )

        load_piece(0)
        load_piece(1)

        ACT_FN = mybir.ActivationFunctionType

        def act_direct(out_ap, in_ap, func, bias=0.0, scale=1.0, alpha=0.0):
            # same lowering as nc.scalar.activation but without the
            # Reciprocal accuracy guard (domain here is ~[1.1, 3.6])
            eng = nc.scalar
            ins = [eng.lower_ap(in_ap)]
            for val in (bias, scale, alpha):
                ins.append(mybir.ImmediateValue(dtype=mybir.dt.float32, value=val))
            return eng.add_instruction(
                mybir.InstActivation(
                    name=nc.get_next_instruction_name(),
                    func=func,
                    ins=ins,
                    outs=[eng.lower_ap(out_ap)],
                )
            )

        # pre-warm the ACT spline tables under the input transfer
        warm = warm_pool.tile([1, 2], F32, tag="warm")
        nc.vector.memset(warm[:], 1.0)
        act_direct(warm[0:1, 1:2], warm[0:1, 0:1], ACT_FN.Reciprocal)

        piece_starts = {PIECE_PAIRS[i][0]: i + 2
                        for i in range(len(PIECE_PAIRS) - 2)}

        def rhs(c):
            return slice(WCOLS + c * CHUNK, WCOLS + (c + 1) * CHUNK)

        # deferred 'finish' closures for staged (B) pairs: the pair-TT and
        # store are emitted one pair LATE so the DVE's strict-FIFO queue
        # never head-of-line blocks on the ACT staging copies
        deferred = []

        def flush_deferred():
            while deferred:
                deferred.pop(0)()

        for j in range(PAIRS):
            if j in piece_starts:
                load_piece(piece_starts[j])

            zt = z_pool.tile([N_VIEWS, 1024], F32, tag="zt")
            rz = rz_pool.tile([N_VIEWS, 2 * CHUNK], F16, tag="rz")
            sb_out = out_pool.tile([N_VIEWS, 4 * CHUNK], F16, tag="g")
            ov = sb_out[:].rearrange("p (two h n) -> p two h n", two=2, h=2)

            # z matmuls FIRST so the pair's reciprocal runs ahead of the
            # TTs that consume it (keeps ACT off the critical path)
            for h in (0, 1):
                nc.tensor.matmul(
                    zt[:, h * 512 : h * 512 + CHUNK],
                    btile[64 : 64 + KR, 0:WCOLS],
                    btile[64 : 64 + KR, rhs(2 * j + h)],
                    tile_position=(64, 0),
                )
            zt3 = zt[:].rearrange("p (b n) -> p b n", b=2)[:, :, 0:CHUNK]
            rz3 = rz[:].rearrange("p (b n) -> p b n", b=2)
            if clamp:
                zcl = sbuv_pool.tile([N_VIEWS, 2 * CHUNK], F32, tag="zcl")
                zcl3 = zcl[:].rearrange("p (b n) -> p b n", b=2)
                nc.vector.tensor_scalar_max(zcl3, zt3, -Z_MAX)
                act_direct(rz3, zcl3, ACT_FN.Reciprocal)
            else:
                act_direct(rz3, zt3, ACT_FN.Reciprocal)

            puvs = []
            for h in (0, 1):
                c = 2 * j + h
                puv = uv_pool.tile([N_VIEWS, 1024], F32, tag="puv")
                puvs.append(puv)
                for g, dst in ((0, puv[:, 0:CHUNK]),
                               (1, puv[:, 512 : 512 + CHUNK])):
                    base = 32 * g
                    nc.tensor.matmul(
                        dst,
                        btile[base : base + KR, 0:WCOLS],
                        btile[base : base + KR, rhs(c)],
                        tile_position=(base, 0),
                    )

            kind = 'A' if clamp else classes[j]

            def store_pair(jj, tile_):
                if jj == PAIRS - 1:
                    # split the final store so the tail only waits on 256 KB
                    half = 2 * CHUNK
                    nc.sync.dma_start(
                        out[:, jj * 4 * CHUNK : jj * 4 * CHUNK + half],
                        tile_[:, 0:half])
                    nc.sync.dma_start(
                        out[:, jj * 4 * CHUNK + half : (jj + 1) * 4 * CHUNK],
                        tile_[:, half : 4 * CHUNK])
                else:
                    nc.sync.dma_start(
                        out[:, jj * 4 * CHUNK : (jj + 1) * 4 * CHUNK],
                        tile_[:])

            if kind == 'A':
                for h in (0, 1):
                    puv4 = (puvs[h][:].rearrange("p (b n) -> p b n", b=2)
                            [:, :, 0:CHUNK].unsqueeze(2))
                    rb = (rz[:, h * CHUNK : (h + 1) * CHUNK]
                          .unsqueeze(1).unsqueeze(1)
                          .broadcast_to([N_VIEWS, 2, 1, CHUNK]))
                    odst = ov[:, :, h : h + 1, :]
                    if clamp:
                        tuv = sbuv_pool.tile([N_VIEWS, 2 * CHUNK], F32,
                                             tag="tuv")
                        t4 = (tuv[:].rearrange("p (b n) -> p b n", b=2)
                              .unsqueeze(2))
                        nc.vector.tensor_tensor(t4, puv4, rb,
                                                mybir.AluOpType.mult)
                        nc.vector.tensor_scalar_add(
                            ov[:, 0:1, h : h + 1, :], t4[:, 0:1, :, :], CX)
                        nc.vector.tensor_scalar_add(
                            ov[:, 1:2, h : h + 1, :], t4[:, 1:2, :, :], CY)
                    else:
                        nc.vector.tensor_tensor(odst, puv4, rb,
                                                mybir.AluOpType.mult)
                store_pair(j, sb_out)
                flush_deferred()
            else:
                # stage both chunks to SBUF fp16 (ACT), then ONE pair-wide
                # TT: [p, uv, h, n] all step-1 fp16 -> DVE 2x_1p
                sbuv = sbuv_pool.tile([N_VIEWS, 4 * CHUNK], F16, tag="sbuv")
                sv = sbuv[:].rearrange("p (two h n) -> p two h n", two=2, h=2)
                for h in (0, 1):
                    puv4 = (puvs[h][:].rearrange("p (b n) -> p b n", b=2)
                            [:, :, 0:CHUNK].unsqueeze(2))
                    act_direct(sv[:, :, h : h + 1, :], puv4, ACT_FN.Copy)
                rbp = (rz[:].rearrange("p (h n) -> p h n", h=2)
                       .unsqueeze(1)
                       .broadcast_to([N_VIEWS, 2, 2, CHUNK]))

                def finish(jj=j, sv=sv, rbp=rbp, ovd=ov, tile_=sb_out,
                           eng=(nc.vector if kind == 'B' else nc.gpsimd)):
                    eng.tensor_tensor(ovd, sv, rbp, mybir.AluOpType.mult)
                    store_pair(jj, tile_)

                deferred.append(finish)

        flush_deferred()

    return _install_wait_legalizer(nc)


_module_cache = {}


def _get_module(clamp):
    if clamp not in _module_cache:
        _module_cache[clamp] = _build_module(clamp)
    return _module_cache[clamp]


# ---------------------------------------------------------------------------
# Entry point
# ---------------------------------------------------------------------------
def kernel(points3d, euler_angles, translations, focal_length, _trace=False):
    import ml_dtypes

    points3d = np.asarray(points3d, dtype=np.float32)
    euler_angles = np.asarray(euler_angles, dtype=np.float32)
    translations = np.asarray(translations, dtype=np.float32)
    focal_length = np.asarray(focal_length, dtype=np.float32)

    # Is the Z clamp provably inactive?  The fast path folds CX/CY into the
    # matmul, which is only exact when no point clamps.
    Rq = _euler_to_matrix(euler_angles.astype(np.float64))
    tz = translations[:, 2].astype(np.float64)
    r2n = np.linalg.norm(Rq[:, 2, :], axis=1)
    pmax = float(np.linalg.norm(points3d.astype(np.float64), axis=1).max())
    znega_lo = float((-tz - r2n * pmax).min())
    clamp = bool(znega_lo < max(-Z_MAX * 10.0, 1e-3))

    Wu, Wv, Wz = _fold_weights(euler_angles, translations, focal_length, clamp)

    pT = points3d.T.astype(ml_dtypes.bfloat16)        # [3, N]
    ones = np.ones((2, N_POINTS), dtype=ml_dtypes.bfloat16)
    pk = np.concatenate([pT, ones], axis=0)           # [KR, N]

    nc = _get_module(clamp)
    in_maps = []
    for c in range(N_CORES):
        sl = pk[:, c * NPC : (c + 1) * NPC]
        in_maps.append(
            {
                "blob": np.ascontiguousarray(
                    np.concatenate([Wu, sl, Wv, sl, Wz, sl], axis=1)
                ),
            }
        )

    res = run_bass_kernel_spmd(
        nc, in_maps, core_ids=list(range(N_CORES)), trace=_trace
    )

    full = np.empty((N_VIEWS, N_POINTS, 2), dtype=np.float32)
    for c in range(N_CORES):
        r = np.asarray(res.results[c]["out"])
        r = r.reshape(N_VIEWS, PAIRS, 2, 2, CHUNK).transpose(0, 1, 3, 4, 2)
        full[:, c * NPC : (c + 1) * NPC, :] = r.reshape(
            N_VIEWS, NPC, 2).astype(np.float32)
    if _trace:
        return full, res
    return full
